# revision 22
# baseline (speedup 1.0000x reference)
"""Bass/Trainium2 multi-head attention kernel, SPMD over 8 NeuronCores.

Problem (nn_MultiHeadAttention):
    x: [8, 1024, 1024] f32; W_split, W_out: [1024, 1024]; Wq/Wk/Wv: [16, 64, 64]
    xp = (x @ W_split.T) -> per-head q/k/v projections -> softmax attention
    -> concat -> @ W_out.T

Sharding: data-parallel over batch (8 batches -> 8 cores), no collectives.

Device algorithm per core (t = 1024 tokens for one batch):
  - xp^T = Ws @ x^T (PE, K=128, 128 matmuls) computed once; Q/K are never
    materialized: the bilinear fold
        scores_h = xp_h (Wq_h^T Wk_h / 8) xp_h^T = xp_h M_h xp_h^T
    needs only th2_h^T = M_h^T xp_h^T, a 64x64-weight matmul per head.
  - V token-major via host-folded WVe (Wv_h folded into W_split per head):
    V[u, feat] = x-block^T-as-lhsT @ WVe^T, exactly v1's layout (bank-
    aligned N=512 outputs; per-head 64-wide matmul outputs would need
    sub-bank PSUM offsets, which abort the hardware). Ones column appended
    per head for the softmax denominator.
  - per head h: S^T[u, s] = th2-block @ xp_h^T (K=64); A = exp(S^T) via ACT
    straight from PSUM (scale folded into M_h; scores ~N(0, 0.01): no
    max-subtraction needed); out_aug^T[o(65), s] = V_aug_h^T @ A accumulated
    over u-blocks, row 64 = denominator; normalize via DVE recip + gpsimd
    partition-broadcast + DVE mul into concat^T.
  - y[t, j] = concat @ W_out^T (PE). W_out^T shares the SBUF slot of the
    (dead) WVe weights; x^T's slot is reused for concat^T.

  Emission interleaves projection work into the attention stream so the
  ACT-bound exp phase starts early and hides the projection tail.
"""

import os
import sys

for _p in ("/opt/trn_rl_repo",):
    if os.path.isdir(_p) and _p not in sys.path:
        sys.path.insert(0, _p)

import numpy as np

import concourse.bass as bass
import concourse.tile as tile
from concourse import bacc, mybir
from concourse.bass import ts
from concourse.bass_utils import run_bass_kernel_spmd

F32 = mybir.dt.float32
F32R = mybir.dt.float32r
BF16 = mybir.dt.bfloat16
# matmul operand dtype: fp32r (11-bit mantissa, 1 cyc/row in sim) or bf16
MM_DT_NAME = os.environ.get("BASS_MM_DT", "fp32r")
MMDT = BF16 if MM_DT_NAME == "bf16" else F32R
N_CORES = 8
B, S, D = 8, 1024, 1024
H, HD = 16, 64
P = 128
KB = D // P  # 8 i-blocks of 128
FB = D // P  # 8 feature-blocks (= head pairs)

EXP = mybir.ActivationFunctionType.Exp


def emit_xp_block(nc, pools, fb, xt_sb, xp_sb, ws_d, wt_tiles=None,
                  ib_range=None, ps_xp=None):
    """xp^T block fb; ib_range/ps_xp allow splitting across filler slots."""
    const, wtile, a_pool, small, av_pool, sps, proj = pools
    if ps_xp is None:
        ps_xp = proj.tile([P, S], F32, tag="ps", name="ps_xp")
    ibs = range(KB) if ib_range is None else ib_range
    for ib in ibs:
        if wt_tiles is not None:
            wt = wt_tiles[ib]
        else:
            wt = wtile.tile([P, P], MMDT, tag="ws")
            (nc.sync if ib % 2 == 0 else nc.gpsimd).dma_start(
                wt[:], ws_d[fb, ib]
            )
        for nh in range(2):
            nc.tensor.matmul(
                ps_xp[:, ts(nh, 512)],
                wt[:],
                xt_sb[:, ib, ts(nh, 512)],
                start=(ib == 0),
                stop=(ib == KB - 1),
            )
    if ib_range is None or ibs[-1] == KB - 1:
        nc.vector.tensor_copy(xp_sb[:, fb, :], ps_xp[:])
    return ps_xp


def emit_th2_pair(nc, pools, fb, xp_sb, th2_sb, mq_sb, h01s=(0, 1)):
    const, wtile, a_pool, small, av_pool, sps, proj = pools
    # odd head's output must land on partitions 0:64 (PE quadrant (64,64)
    # is unsupported), so one PSUM tile per head
    for h01 in h01s:
        pq = h01 * HD
        ps_t = proj.tile([HD, S], F32, tag="ps", name=f"ps_t{h01}")
        for nh in range(2):
            nc.tensor.matmul(
                ps_t[:, ts(nh, 512)],
                mq_sb[pq : pq + HD, fb, :],
                xp_sb[pq : pq + HD, fb, ts(nh, 512)],
                start=True,
                stop=True,
            )
        nc.vector.tensor_copy(th2_sb[pq : pq + HD, fb, :], ps_t[:])


def emit_v_block(nc, pools, tb, xt_sb, wvt_sb, vaug_sb):
    """V token-major for token block tb, all heads: V = x @ WVe^T."""
    const, wtile, a_pool, small, av_pool, sps, proj = pools
    ps = proj.tile([P, D], F32, tag="ps", name="ps_v")
    for kb in range(KB):
        for nh in range(2):
            nc.tensor.matmul(
                ps[:, ts(nh, 512)],
                xt_sb[:, kb, ts(tb, P)],
                wvt_sb[:, kb, ts(nh, 512)],
                start=(kb == 0),
                stop=(kb == KB - 1),
            )
    # scatter heads into the ones-augmented layout (stride HD+1)
    nc.vector.tensor_copy(
        vaug_sb[:, tb, :, 0:HD],
        ps[:].rearrange("p (h o) -> p h o", h=H),
    )


def emit_attn_head(nc, pools, h, xp_sb, th2_sb, vaug_sb, concat_sb,
                   pe_filler=None):
    """Attention for head h; PSUM: av (2 banks) + s_ps rotating.

    pe_filler: optional callable(ub) emitting extra PE work between the
    score matmuls and AV matmuls of each u-block (used to weave projection
    blocks into the stream without starving ACT).
    """
    const, wtile, a_pool, small, av_pool, sps, proj = pools
    fb = h // 2
    pq = (h % 2) * HD
    av = av_pool.tile([P, S], F32, tag="av", name=f"av{h}")

    def emit_scores(ub):
        s_ps = sps.tile([P, S], F32, tag="sps", name="s_ps")
        for nh in range(2):
            nc.tensor.matmul(
                s_ps[:, ts(nh, 512)],
                th2_sb[pq : pq + HD, fb, ts(ub, P)],
                xp_sb[pq : pq + HD, fb, ts(nh, 512)],
                start=True,
                stop=True,
            )
        a_sb = a_pool.tile([P, S], MMDT, tag="a")
        nc.scalar.activation(a_sb[:], s_ps[:], EXP, scale=1.0)
        return a_sb

    # software pipeline: scores run one u-block ahead of AV so the PE never
    # sits behind an exp in its own FIFO (each cross-engine edge costs real
    # latency on hw)
    a_cur = emit_scores(0)
    for ub in range(KB):
        a_next = emit_scores(ub + 1) if ub < KB - 1 else None
        if pe_filler is not None:
            pe_filler(ub)
        vt = vaug_sb[:, ub, h, :]  # [128, 65]
        for nh in range(2):
            nc.tensor.matmul(
                av[0 : HD + 1, ts(nh, 512)],
                vt,
                a_cur[:, ts(nh, 512)],
                start=(ub == 0),
                stop=(ub == KB - 1),
            )
        a_cur = a_next
    # free the av PSUM slot with a single copy; normalize runs from SBUF
    # off the critical path (only phase C depends on concat)
    av_sb = small.tile([HD + 1, S], F32R, tag="av_sb")
    nc.vector.tensor_copy(av_sb[:], av[0 : HD + 1, :])
    recip = small.tile([1, S], F32R, tag="recip")
    with nc.allow_low_precision(reason="fp32r 12-bit mantissa; 1e-4 rel ok"):
        nc.vector.reciprocal(recip[:], av_sb[HD : HD + 1, :])
    bc_sb = small.tile([HD, S], F32R, tag="bc")
    nc.gpsimd.partition_broadcast(bc_sb[:], recip[:])
    nc.vector.tensor_mul(
        concat_sb[pq : pq + HD, fb, :],
        av_sb[0:HD, :],
        bc_sb[:],
    )


def emit_body(nc, tc, pools, dram, phases=("proj", "attn", "final")):
    const, wtile, a_pool, small, av_pool, sps, proj = pools
    xt_d, ws_d, mq_d, wvt_d, wout_d, y_d = dram

    if "noop" in phases:
        tiny = small.tile([P, 64], F32, tag="tiny")
        nc.gpsimd.memset(tiny[:], 0.0)
        return

    # ---- resident SBUF tensors ----
    xt_sb = const.tile([P, KB, S], MMDT, tag="big_a")        # x^T  [i, t]
    xp_sb = const.tile([P, FB, S], MMDT, tag="xp")           # xp^T [feat, t]
    th2_sb = const.tile([P, FB, S], MMDT, tag="th2")         # th2^T pairs
    vaug_sb = const.tile([P, KB, H, HD + 1], MMDT, tag="vaug")
    wvt_sb = const.tile([P, KB, D], MMDT, tag="big_b")       # WVe^T [i, feat]
    mq_sb = const.tile([P, FB, HD], MMDT, tag="mq")          # mqT pairs [d, d']
    # memset can't write fp32r; stage in f32 and convert via DVE copy
    ones_f32 = small.tile([P, KB * H], F32, tag="ones_f32")
    nc.gpsimd.memset(ones_f32[:], 1.0)
    nc.vector.tensor_copy(vaug_sb[:, :, :, HD : HD + 1], ones_f32[:])

    # Startup DMA: interleave x^T chunks with xp-block-0's weight tiles
    # across sync+gpsimd+scalar (ACT idle until the first exp) so the first
    # projection matmuls start ~2us in; WVe^T trails on the same queues
    # (needed by V in head 0, ~13us in).
    nc.sync.dma_start(mq_sb[:], mq_d[:])
    qs = (nc.sync, nc.gpsimd, nc.scalar)
    wt0 = []
    for ib in range(KB):
        q = qs[ib % 3]
        q.dma_start(xt_sb[:, ib, :], xt_d[ib])
        wt = wtile.tile([P, P], MMDT, tag="ws")
        q.dma_start(wt[:], ws_d[0, ib])
        wt0.append(wt)
    for ib in range(KB):
        qs[ib % 3].dma_start(wvt_sb[:, ib, :], wvt_d[ib])

    do_attn = "attn" in phases

    # xp block 0 + th2 pair 0 first so attention can start ASAP
    emit_xp_block(nc, pools, 0, xt_sb, xp_sb, ws_d, wt_tiles=wt0)
    emit_th2_pair(nc, pools, 0, xp_sb, th2_sb, mq_sb)

    if not do_attn:
        for tb in range(KB):
            emit_v_block(nc, pools, tb, xt_sb, wvt_sb, vaug_sb)
        for fb in range(1, FB):
            emit_xp_block(nc, pools, fb, xt_sb, xp_sb, ws_d)
            emit_th2_pair(nc, pools, fb, xp_sb, th2_sb, mq_sb)
        return

    # concat^T reuses th2's storage range-exactly: head h's th2 rows
    # [pq:pq+64, fb] are dead once its scores finish, which is exactly when
    # its normalize writes concat[pq:pq+64, fb]
    concat_sb = th2_sb
    wout_sb = const.tile([P, KB, D], MMDT, tag="big_b")    # after last wvt read

    # head 0 weaves the V blocks (AV of u-block ub needs V of token block
    # ub, emitted just-in-time); each odd head weaves the next pair's
    # xp+th2 at its second u-block so the pair boundary has no dependency
    # stall; wout's DMAs ride sync after head 0's emission
    def v_filler(ub):
        emit_v_block(nc, pools, ub, xt_sb, wvt_sb, vaug_sb)

    def make_proj_filler(next_fb):
        state = {}

        def filler(ub):
            if ub == 1:
                state["ps"] = emit_xp_block(
                    nc, pools, next_fb, xt_sb, xp_sb, ws_d,
                    ib_range=range(0, 4),
                )
            elif ub == 2:
                emit_xp_block(
                    nc, pools, next_fb, xt_sb, xp_sb, ws_d,
                    ib_range=range(4, 8), ps_xp=state["ps"],
                )
            elif ub == 3:
                emit_th2_pair(nc, pools, next_fb, xp_sb, th2_sb, mq_sb, (0,))
            elif ub == 4:
                emit_th2_pair(nc, pools, next_fb, xp_sb, th2_sb, mq_sb, (1,))

        return filler

    for h in range(H):
        if h == 0:
            filler = v_filler
        elif h % 2 == 1 and h < H - 1:
            filler = make_proj_filler(h // 2 + 1)
        else:
            filler = None
        emit_attn_head(
            nc, pools, h, xp_sb, th2_sb, vaug_sb, concat_sb, pe_filler=filler
        )
        if h == 0:
            for ib in range(KB):
                nc.sync.dma_start(wout_sb[:, ib, :], wout_d[ib])

    if "final" not in phases:
        return
    # ---- phase C: y[t, j] = concat @ W_out^T ----
    for tb in range(KB):
        ps = sps.tile([P, D], F32, tag="sps", name="ps_y")
        for cb in range(KB):
            for nh in range(2):
                nc.tensor.matmul(
                    ps[:, ts(nh, 512)],
                    concat_sb[:, cb, ts(tb, P)],
                    wout_sb[:, cb, ts(nh, 512)],
                    start=(cb == 0),
                    stop=(cb == KB - 1),
                )
        out_sb = a_pool.tile([P, D], F32, tag="a")
        nc.vector.tensor_copy(out_sb[:], ps[:])
        nc.sync.dma_start(y_d[ts(tb, P), :], out_sb[:])


def build_nc(reps: int = 1, phases=("proj", "attn", "final")):
    nc = bacc.Bacc(
        "TRN2", target_bir_lowering=False, debug=False, num_devices=N_CORES
    )
    xt_d = nc.dram_tensor("xt", [KB, P, S], MMDT, kind="ExternalInput")
    ws_d = nc.dram_tensor("ws", [FB, KB, P, P], MMDT, kind="ExternalInput")
    mq_d = nc.dram_tensor("mq", [P, FB, HD], MMDT, kind="ExternalInput")
    wvt_d = nc.dram_tensor("wvt", [KB, P, D], MMDT, kind="ExternalInput")
    wout_d = nc.dram_tensor("wout", [KB, P, D], MMDT, kind="ExternalInput")
    y_d = nc.dram_tensor("y", [S, D], F32, kind="ExternalOutput")
    dram = (xt_d, ws_d, mq_d, wvt_d, wout_d, y_d)

    with tile.TileContext(nc) as tc:
        with (
            tc.tile_pool(name="const", bufs=1) as const,
            tc.tile_pool(name="wtile", bufs=4) as wtile,
            tc.tile_pool(name="a", bufs=4) as a_pool,
            tc.tile_pool(name="small", bufs=2) as small,
            tc.tile_pool(name="av", bufs=1, space="PSUM") as av_pool,
            tc.tile_pool(name="sps", bufs=2, space="PSUM") as sps,
            tc.tile_pool(name="proj", bufs=1, space="PSUM") as proj,
        ):
            pools = (const, wtile, a_pool, small, av_pool, sps, proj)
            if reps == 1:
                emit_body(nc, tc, pools, dram, phases)
            else:
                with tc.For_i(0, reps, 1):
                    emit_body(nc, tc, pools, dram, phases)
    nc.compile()
    return nc


def to_fp32r(a):
    """Round fp32 to fp32r (11-bit mantissa, round-to-nearest-even).

    The PE consumes fp32r at 1 cycle/row (vs 4 for fp32); walrus requires
    fp32r matmul operands to be pre-rounded.
    """
    v = np.ascontiguousarray(a, np.float32).view(np.uint32).astype(np.uint64)
    lsb = (v >> 12) & 1
    v = (v + 0x7FF + lsb) & ~np.uint64(0xFFF)
    return v.astype(np.uint32).view(np.float32)


def to_mm(a):
    """Round fp32 to the matmul operand dtype (fp32r or bf16)."""
    if MM_DT_NAME == "bf16":
        import ml_dtypes

        return np.ascontiguousarray(a, np.float32).astype(ml_dtypes.bfloat16)
    return to_fp32r(a)


def prep_inputs(x, W_split, W_out, Wq, Wk, Wv):
    """Host-side layout prep + weight folds. Per-core input maps."""
    x = np.asarray(x, np.float32)
    Ws = np.asarray(W_split, np.float64)
    Wq = np.asarray(Wq, np.float64)
    Wk = np.asarray(Wk, np.float64)
    Wv = np.asarray(Wv, np.float64)

    # Ws^T tiles: lhsT for xp^T = Ws @ x^T -> lhsT[i, f] = W_split^T
    ws_tiles = to_mm(
        np.ascontiguousarray(
            Ws.T.reshape(KB, P, FB, P).transpose(2, 0, 1, 3), np.float32
        )
    )  # [fb, ib, 128, 128]

    # mqT_h = (Wq_h^T Wk_h / 8)^T = Wk_h^T Wq_h / 8, pairs stacked on rows
    mqT = np.einsum("hod,hoe->hde", Wk, Wq) / np.sqrt(np.float64(HD))
    mq_pack = to_mm(
        np.ascontiguousarray(
            mqT.reshape(FB, 2 * HD, HD).transpose(1, 0, 2), np.float32
        )
    )  # [128, fb, 64]

    # WVe: per-head Wv folded into W_split -> V = x @ WVe^T, token-major
    Wsh = Ws.reshape(H, HD, D)  # [h, d, i]
    WVe = np.einsum("hod,hdi->hoi", Wv, Wsh).reshape(D, D)
    wvt = to_mm(np.ascontiguousarray(WVe.T.reshape(KB, P, D), np.float32))

    woutT = to_mm(
        np.ascontiguousarray(
            np.asarray(W_out, np.float64).T.reshape(KB, P, D), np.float32
        )
    )

    in_maps = []
    for b in range(B):
        xt = to_mm(np.ascontiguousarray(x[b].T.reshape(KB, P, S)))
        in_maps.append(
            {"xt": xt, "ws": ws_tiles, "mq": mq_pack, "wvt": wvt, "wout": woutT}
        )
    return in_maps


_NC_CACHE = {}


def kernel(x, W_split, W_out, Wq, Wk, Wv):
    if "nc" not in _NC_CACHE:
        _NC_CACHE["nc"] = build_nc(reps=1)
    nc = _NC_CACHE["nc"]
    in_maps = prep_inputs(x, W_split, W_out, Wq, Wk, Wv)
    res = run_bass_kernel_spmd(nc, in_maps, list(range(N_CORES)))
    out = np.stack([res.results[b]["y"] for b in range(B)], axis=0)
    return out.astype(np.float32)


if __name__ == "__main__":
    rng = np.random.default_rng(0)
    inputs = {
        "x": rng.standard_normal((B, S, D)).astype(np.float32),
        "W_split": (rng.standard_normal((D, D)) * 0.02).astype(np.float32),
        "W_out": (rng.standard_normal((D, D)) * 0.02).astype(np.float32),
        "Wq": (rng.standard_normal((H, HD, HD)) * 0.02).astype(np.float32),
        "Wk": (rng.standard_normal((H, HD, HD)) * 0.02).astype(np.float32),
        "Wv": (rng.standard_normal((H, HD, HD)) * 0.02).astype(np.float32),
    }
    y = kernel(**inputs)
    print("kernel output:", y.shape, y.dtype, np.abs(y).max())


# revision 23
# speedup vs baseline: 1.6798x; 1.6798x over previous
"""Bass/Trainium2 multi-head attention kernel, SPMD over 8 NeuronCores.

Problem (nn_MultiHeadAttention):
    x: [8, 1024, 1024] f32; W_split, W_out: [1024, 1024]; Wq/Wk/Wv: [16, 64, 64]
    xp = (x @ W_split.T) -> per-head q/k/v projections -> softmax attention
    -> concat -> @ W_out.T

Sharding: data-parallel over batch (8 batches -> 8 cores), no collectives.

Device algorithm per core (t = 1024 tokens for one batch):
  - xp^T = Ws @ x^T (PE, K=128, 128 matmuls) computed once; Q/K are never
    materialized: the bilinear fold
        scores_h = xp_h (Wq_h^T Wk_h / 8) xp_h^T = xp_h M_h xp_h^T
    needs only th2_h^T = M_h^T xp_h^T, a 64x64-weight matmul per head.
  - V token-major via host-folded WVe (Wv_h folded into W_split per head):
    V[u, feat] = x-block^T-as-lhsT @ WVe^T, exactly v1's layout (bank-
    aligned N=512 outputs; per-head 64-wide matmul outputs would need
    sub-bank PSUM offsets, which abort the hardware). Ones column appended
    per head for the softmax denominator.
  - per head h: S^T[u, s] = th2-block @ xp_h^T (K=64); A = exp(S^T) via ACT
    straight from PSUM (scale folded into M_h; scores ~N(0, 0.01): no
    max-subtraction needed); out_aug^T[o(65), s] = V_aug_h^T @ A accumulated
    over u-blocks, row 64 = denominator; normalize via DVE recip + gpsimd
    partition-broadcast + DVE mul into concat^T.
  - y[t, j] = concat @ W_out^T (PE). W_out^T shares the SBUF slot of the
    (dead) WVe weights; x^T's slot is reused for concat^T.

  Emission interleaves projection work into the attention stream so the
  ACT-bound exp phase starts early and hides the projection tail.
"""

import os
import sys

for _p in ("/opt/trn_rl_repo",):
    if os.path.isdir(_p) and _p not in sys.path:
        sys.path.insert(0, _p)

import numpy as np

import concourse.bass as bass
import concourse.tile as tile
from concourse import bacc, mybir
from concourse.bass import ts
from concourse.bass_utils import run_bass_kernel_spmd

F32 = mybir.dt.float32
F32R = mybir.dt.float32r
BF16 = mybir.dt.bfloat16
# matmul operand dtype: fp32r (11-bit mantissa, 1 cyc/row in sim) or bf16
MM_DT_NAME = os.environ.get("BASS_MM_DT", "fp32r")
MMDT = BF16 if MM_DT_NAME == "bf16" else F32R
N_CORES = 8
B, S, D = 8, 1024, 1024
H, HD = 16, 64
P = 128
KB = D // P  # 8 i-blocks of 128
FB = D // P  # 8 feature-blocks (= head pairs)

EXP = mybir.ActivationFunctionType.Exp


def emit_xp_block(nc, pools, fb, xt_sb, xp_sb, ws_d, wt_tiles=None,
                  ib_range=None, ps_xp=None):
    """xp^T block fb; ib_range/ps_xp allow splitting across filler slots."""
    const, wtile, a_pool, small, av_pool, sps, proj = pools
    if ps_xp is None:
        ps_xp = proj.tile([P, S], F32, tag="ps", name="ps_xp")
    ibs = range(KB) if ib_range is None else ib_range
    for ib in ibs:
        if wt_tiles is not None:
            wt = wt_tiles[ib]
        else:
            wt = wtile.tile([P, P], MMDT, tag="ws")
            (nc.sync if ib % 2 == 0 else nc.gpsimd).dma_start(
                wt[:], ws_d[fb, ib]
            )
        for nh in range(2):
            nc.tensor.matmul(
                ps_xp[:, ts(nh, 512)],
                wt[:],
                xt_sb[:, ib, ts(nh, 512)],
                start=(ib == 0),
                stop=(ib == KB - 1),
            )
    if ib_range is None or ibs[-1] == KB - 1:
        nc.vector.tensor_copy(xp_sb[:, fb, :], ps_xp[:])
    return ps_xp


def emit_th2_pair(nc, pools, fb, xp_sb, th2_sb, mq_sb, h01s=(0, 1)):
    const, wtile, a_pool, small, av_pool, sps, proj = pools
    # odd head's output must land on partitions 0:64 (PE quadrant (64,64)
    # is unsupported), so one PSUM tile per head
    for h01 in h01s:
        pq = h01 * HD
        ps_t = proj.tile([HD, S], F32, tag="ps", name=f"ps_t{h01}")
        for nh in range(2):
            nc.tensor.matmul(
                ps_t[:, ts(nh, 512)],
                mq_sb[pq : pq + HD, fb, :],
                xp_sb[pq : pq + HD, fb, ts(nh, 512)],
                start=True,
                stop=True,
            )
        nc.vector.tensor_copy(th2_sb[pq : pq + HD, fb, :], ps_t[:])


def emit_v_block(nc, pools, tb, xt_sb, wvt_sb, vaug_sb):
    """V token-major for token block tb, all heads: V = x @ WVe^T."""
    const, wtile, a_pool, small, av_pool, sps, proj = pools
    ps = proj.tile([P, D], F32, tag="ps", name="ps_v")
    for kb in range(KB):
        for nh in range(2):
            nc.tensor.matmul(
                ps[:, ts(nh, 512)],
                xt_sb[:, kb, ts(tb, P)],
                wvt_sb[:, kb, ts(nh, 512)],
                start=(kb == 0),
                stop=(kb == KB - 1),
            )
    # scatter heads into the ones-augmented layout (stride HD+1)
    nc.vector.tensor_copy(
        vaug_sb[:, tb, :, 0:HD],
        ps[:].rearrange("p (h o) -> p h o", h=H),
    )


def emit_attn_head(nc, pools, h, xp_sb, th2_sb, vaug_sb, concat_sb,
                   pe_filler=None):
    """Attention for head h; PSUM: av (2 banks) + s_ps rotating.

    pe_filler: optional callable(ub) emitting extra PE work between the
    score matmuls and AV matmuls of each u-block (used to weave projection
    blocks into the stream without starving ACT).
    """
    const, wtile, a_pool, small, av_pool, sps, proj = pools
    fb = h // 2
    pq = (h % 2) * HD
    av = av_pool.tile([P, S], F32, tag="av", name=f"av{h}")

    def emit_scores(ub):
        s_ps = sps.tile([P, S], F32, tag="sps", name="s_ps")
        for nh in range(2):
            nc.tensor.matmul(
                s_ps[:, ts(nh, 512)],
                th2_sb[pq : pq + HD, fb, ts(ub, P)],
                xp_sb[pq : pq + HD, fb, ts(nh, 512)],
                start=True,
                stop=True,
            )
        a_sb = a_pool.tile([P, S], MMDT, tag="a")
        nc.scalar.activation(a_sb[:], s_ps[:], EXP, scale=1.0)
        return a_sb

    # software pipeline: scores run one u-block ahead of AV so the PE never
    # sits behind an exp in its own FIFO (each cross-engine edge costs real
    # latency on hw)
    a_cur = emit_scores(0)
    for ub in range(KB):
        a_next = emit_scores(ub + 1) if ub < KB - 1 else None
        if pe_filler is not None:
            pe_filler(ub)
        vt = vaug_sb[:, ub, h, :]  # [128, 65]
        for nh in range(2):
            nc.tensor.matmul(
                av[0 : HD + 1, ts(nh, 512)],
                vt,
                a_cur[:, ts(nh, 512)],
                start=(ub == 0),
                stop=(ub == KB - 1),
            )
        a_cur = a_next
    # free the av PSUM slot with a single copy; normalize runs from SBUF
    # off the critical path (only phase C depends on concat)
    av_sb = small.tile([HD + 1, S], F32R, tag="av_sb")
    nc.vector.tensor_copy(av_sb[:], av[0 : HD + 1, :])
    recip = small.tile([1, S], F32R, tag="recip")
    with nc.allow_low_precision(reason="fp32r 12-bit mantissa; 1e-4 rel ok"):
        nc.vector.reciprocal(recip[:], av_sb[HD : HD + 1, :])
    bc_sb = small.tile([HD, S], F32R, tag="bc")
    nc.gpsimd.partition_broadcast(bc_sb[:], recip[:])
    nc.vector.tensor_mul(
        concat_sb[pq : pq + HD, fb, :],
        av_sb[0:HD, :],
        bc_sb[:],
    )


def emit_body(nc, tc, pools, dram, phases=("proj", "attn", "final")):
    const, wtile, a_pool, small, av_pool, sps, proj = pools
    xt_d, ws_d, mq_d, wvt_d, wout_d, y_d = dram

    if "noop" in phases:
        tiny = small.tile([P, 64], F32, tag="tiny")
        nc.gpsimd.memset(tiny[:], 0.0)
        return

    if "actonly" in phases:
        # pure ACT throughput probe: 128 dependency-free exps of N=1024
        src = const.tile([P, S], F32, tag="xp")
        nc.gpsimd.memset(src[:], 0.0)
        for _ in range(H * KB):
            a_sb = a_pool.tile([P, S], MMDT, tag="a")
            nc.scalar.activation(a_sb[:], src[:], EXP, scale=1.0)
        return

    # ---- resident SBUF tensors ----
    xt_sb = const.tile([P, KB, S], MMDT, tag="big_a")        # x^T  [i, t]
    xp_sb = const.tile([P, FB, S], MMDT, tag="xp")           # xp^T [feat, t]
    th2_sb = const.tile([P, FB, S], MMDT, tag="th2")         # th2^T pairs
    vaug_sb = const.tile([P, KB, H, HD + 1], MMDT, tag="vaug")
    wvt_sb = const.tile([P, KB, D], MMDT, tag="big_b")       # WVe^T [i, feat]
    mq_sb = const.tile([P, FB, HD], MMDT, tag="mq")          # mqT pairs [d, d']
    # memset can't write fp32r; stage in f32 and convert via DVE copy
    ones_f32 = small.tile([P, KB * H], F32, tag="ones_f32")
    nc.gpsimd.memset(ones_f32[:], 1.0)
    nc.vector.tensor_copy(vaug_sb[:, :, :, HD : HD + 1], ones_f32[:])

    if "attnonly" in phases:
        # attention timing probe: zero inputs (exp(0)=1, all finite)
        z32 = small.tile([P, S], F32, tag="z32")
        nc.gpsimd.memset(z32[:], 0.0)
        for fb in range(FB):
            nc.vector.tensor_copy(xp_sb[:, fb, :], z32[:])
            nc.vector.tensor_copy(th2_sb[:, fb, :], z32[:])
        for tb in range(KB):
            nc.vector.tensor_copy(
                vaug_sb[:, tb, :, 0:HD],
                z32[:].rearrange("p (h o) -> p h o", h=H),
            )
        for h in range(H):
            emit_attn_head(nc, pools, h, xp_sb, th2_sb, vaug_sb, th2_sb)
        return

    # Startup DMA: interleave x^T chunks with xp-block-0's weight tiles
    # across sync+gpsimd+scalar (ACT idle until the first exp) so the first
    # projection matmuls start ~2us in; WVe^T trails on the same queues
    # (needed by V in head 0, ~13us in).
    nc.sync.dma_start(mq_sb[:], mq_d[:])
    qs = (nc.sync, nc.gpsimd, nc.scalar)
    wt0 = []
    for ib in range(KB):
        q = qs[ib % 3]
        q.dma_start(xt_sb[:, ib, :], xt_d[ib])
        wt = wtile.tile([P, P], MMDT, tag="ws")
        q.dma_start(wt[:], ws_d[0, ib])
        wt0.append(wt)
    for ib in range(KB):
        qs[ib % 3].dma_start(wvt_sb[:, ib, :], wvt_d[ib])

    do_attn = "attn" in phases

    # xp block 0 + th2 pair 0 first so attention can start ASAP
    emit_xp_block(nc, pools, 0, xt_sb, xp_sb, ws_d, wt_tiles=wt0)
    emit_th2_pair(nc, pools, 0, xp_sb, th2_sb, mq_sb)

    if not do_attn:
        for tb in range(KB):
            emit_v_block(nc, pools, tb, xt_sb, wvt_sb, vaug_sb)
        for fb in range(1, FB):
            emit_xp_block(nc, pools, fb, xt_sb, xp_sb, ws_d)
            emit_th2_pair(nc, pools, fb, xp_sb, th2_sb, mq_sb)
        return

    # concat^T reuses th2's storage range-exactly: head h's th2 rows
    # [pq:pq+64, fb] are dead once its scores finish, which is exactly when
    # its normalize writes concat[pq:pq+64, fb]
    concat_sb = th2_sb
    wout_sb = const.tile([P, KB, D], MMDT, tag="big_b")    # after last wvt read

    # head 0 weaves the V blocks (AV of u-block ub needs V of token block
    # ub, emitted just-in-time); each odd head weaves the next pair's
    # xp+th2 at its second u-block so the pair boundary has no dependency
    # stall; wout's DMAs ride sync after head 0's emission
    def v_filler(ub):
        emit_v_block(nc, pools, ub, xt_sb, wvt_sb, vaug_sb)

    def make_proj_filler(next_fb):
        state = {}

        def filler(ub):
            if ub == 1:
                state["ps"] = emit_xp_block(
                    nc, pools, next_fb, xt_sb, xp_sb, ws_d,
                    ib_range=range(0, 4),
                )
            elif ub == 2:
                emit_xp_block(
                    nc, pools, next_fb, xt_sb, xp_sb, ws_d,
                    ib_range=range(4, 8), ps_xp=state["ps"],
                )
            elif ub == 3:
                emit_th2_pair(nc, pools, next_fb, xp_sb, th2_sb, mq_sb, (0,))
            elif ub == 4:
                emit_th2_pair(nc, pools, next_fb, xp_sb, th2_sb, mq_sb, (1,))

        return filler

    for h in range(H):
        if h == 0:
            filler = v_filler
        elif h % 2 == 1 and h < H - 1:
            filler = make_proj_filler(h // 2 + 1)
        else:
            filler = None
        emit_attn_head(
            nc, pools, h, xp_sb, th2_sb, vaug_sb, concat_sb, pe_filler=filler
        )
        if h == 0:
            for ib in range(KB):
                nc.sync.dma_start(wout_sb[:, ib, :], wout_d[ib])

    if "final" not in phases:
        return
    # ---- phase C: y[t, j] = concat @ W_out^T ----
    for tb in range(KB):
        ps = sps.tile([P, D], F32, tag="sps", name="ps_y")
        for cb in range(KB):
            for nh in range(2):
                nc.tensor.matmul(
                    ps[:, ts(nh, 512)],
                    concat_sb[:, cb, ts(tb, P)],
                    wout_sb[:, cb, ts(nh, 512)],
                    start=(cb == 0),
                    stop=(cb == KB - 1),
                )
        out_sb = a_pool.tile([P, D], F32, tag="a")
        nc.vector.tensor_copy(out_sb[:], ps[:])
        nc.sync.dma_start(y_d[ts(tb, P), :], out_sb[:])


def build_nc(reps: int = 1, phases=("proj", "attn", "final")):
    nc = bacc.Bacc(
        "TRN2", target_bir_lowering=False, debug=False, num_devices=N_CORES
    )
    xt_d = nc.dram_tensor("xt", [KB, P, S], MMDT, kind="ExternalInput")
    ws_d = nc.dram_tensor("ws", [FB, KB, P, P], MMDT, kind="ExternalInput")
    mq_d = nc.dram_tensor("mq", [P, FB, HD], MMDT, kind="ExternalInput")
    wvt_d = nc.dram_tensor("wvt", [KB, P, D], MMDT, kind="ExternalInput")
    wout_d = nc.dram_tensor("wout", [KB, P, D], MMDT, kind="ExternalInput")
    y_d = nc.dram_tensor("y", [S, D], F32, kind="ExternalOutput")
    dram = (xt_d, ws_d, mq_d, wvt_d, wout_d, y_d)

    with tile.TileContext(nc) as tc:
        with (
            tc.tile_pool(name="const", bufs=1) as const,
            tc.tile_pool(name="wtile", bufs=4) as wtile,
            tc.tile_pool(name="a", bufs=4) as a_pool,
            tc.tile_pool(name="small", bufs=2) as small,
            tc.tile_pool(name="av", bufs=1, space="PSUM") as av_pool,
            tc.tile_pool(name="sps", bufs=2, space="PSUM") as sps,
            tc.tile_pool(name="proj", bufs=1, space="PSUM") as proj,
        ):
            pools = (const, wtile, a_pool, small, av_pool, sps, proj)
            if reps == 1:
                emit_body(nc, tc, pools, dram, phases)
            else:
                with tc.For_i(0, reps, 1):
                    emit_body(nc, tc, pools, dram, phases)
    nc.compile()
    return nc


def to_fp32r(a):
    """Round fp32 to fp32r (11-bit mantissa, round-to-nearest-even).

    The PE consumes fp32r at 1 cycle/row (vs 4 for fp32); walrus requires
    fp32r matmul operands to be pre-rounded.
    """
    v = np.ascontiguousarray(a, np.float32).view(np.uint32).astype(np.uint64)
    lsb = (v >> 12) & 1
    v = (v + 0x7FF + lsb) & ~np.uint64(0xFFF)
    return v.astype(np.uint32).view(np.float32)


def to_mm(a):
    """Round fp32 to the matmul operand dtype (fp32r or bf16)."""
    if MM_DT_NAME == "bf16":
        import ml_dtypes

        return np.ascontiguousarray(a, np.float32).astype(ml_dtypes.bfloat16)
    return to_fp32r(a)


def prep_inputs(x, W_split, W_out, Wq, Wk, Wv):
    """Host-side layout prep + weight folds. Per-core input maps."""
    x = np.asarray(x, np.float32)
    Ws = np.asarray(W_split, np.float64)
    Wq = np.asarray(Wq, np.float64)
    Wk = np.asarray(Wk, np.float64)
    Wv = np.asarray(Wv, np.float64)

    # Ws^T tiles: lhsT for xp^T = Ws @ x^T -> lhsT[i, f] = W_split^T
    ws_tiles = to_mm(
        np.ascontiguousarray(
            Ws.T.reshape(KB, P, FB, P).transpose(2, 0, 1, 3), np.float32
        )
    )  # [fb, ib, 128, 128]

    # mqT_h = (Wq_h^T Wk_h / 8)^T = Wk_h^T Wq_h / 8, pairs stacked on rows
    mqT = np.einsum("hod,hoe->hde", Wk, Wq) / np.sqrt(np.float64(HD))
    mq_pack = to_mm(
        np.ascontiguousarray(
            mqT.reshape(FB, 2 * HD, HD).transpose(1, 0, 2), np.float32
        )
    )  # [128, fb, 64]

    # WVe: per-head Wv folded into W_split -> V = x @ WVe^T, token-major
    Wsh = Ws.reshape(H, HD, D)  # [h, d, i]
    WVe = np.einsum("hod,hdi->hoi", Wv, Wsh).reshape(D, D)
    wvt = to_mm(np.ascontiguousarray(WVe.T.reshape(KB, P, D), np.float32))

    woutT = to_mm(
        np.ascontiguousarray(
            np.asarray(W_out, np.float64).T.reshape(KB, P, D), np.float32
        )
    )

    in_maps = []
    for b in range(B):
        xt = to_mm(np.ascontiguousarray(x[b].T.reshape(KB, P, S)))
        in_maps.append(
            {"xt": xt, "ws": ws_tiles, "mq": mq_pack, "wvt": wvt, "wout": woutT}
        )
    return in_maps


_NC_CACHE = {}


def kernel(x, W_split, W_out, Wq, Wk, Wv):
    if "nc" not in _NC_CACHE:
        _NC_CACHE["nc"] = build_nc(reps=1)
    nc = _NC_CACHE["nc"]
    in_maps = prep_inputs(x, W_split, W_out, Wq, Wk, Wv)
    res = run_bass_kernel_spmd(nc, in_maps, list(range(N_CORES)))
    out = np.stack([res.results[b]["y"] for b in range(B)], axis=0)
    return out.astype(np.float32)


if __name__ == "__main__":
    rng = np.random.default_rng(0)
    inputs = {
        "x": rng.standard_normal((B, S, D)).astype(np.float32),
        "W_split": (rng.standard_normal((D, D)) * 0.02).astype(np.float32),
        "W_out": (rng.standard_normal((D, D)) * 0.02).astype(np.float32),
        "Wq": (rng.standard_normal((H, HD, HD)) * 0.02).astype(np.float32),
        "Wk": (rng.standard_normal((H, HD, HD)) * 0.02).astype(np.float32),
        "Wv": (rng.standard_normal((H, HD, HD)) * 0.02).astype(np.float32),
    }
    y = kernel(**inputs)
    print("kernel output:", y.shape, y.dtype, np.abs(y).max())
